# revision 5
# baseline (speedup 1.0000x reference)
"""Trainium2 Bass kernel for the CustomJacobiLayer problem.

Computes out[b,j] = sum_{i,d} P_d(tanh(x[b,i])) * coef[j,i,d]
with P_d the Jacobi(alpha=1,beta=1) polynomials, d=0..7.

Strategy (8 NeuronCores, data-parallel over batch):
  - Each core owns 512 of the 4096 batch rows; coef is replicated.
  - Host-side: the three-term Jacobi recurrence
        p_d = K1_d * t * p_{d-1} - K3_d * p_{d-2}     (K2_d == 0 for a==b)
    is rescaled with q_d = p_d / s_d, s_d = K1_d * s_{d-1}, so the device
    recurrence has a unit leading coefficient:
        q_d = t * q_{d-1} - g_d * q_{d-2}
    The scales s_d are folded into coef (in float64), so only two fp16
    VectorE ops per order are needed on-chip.
  - The d=0 term is P_0 == 1, i.e. a rank-1 bias sum_i coef[j,i,0]; it is
    computed on the host and added after the gather.
  - Device: ScalarE tanh (f32 -> fp16), VectorE recurrence chain (fp16,
    2x perf mode), 112 accumulating TensorE matmuls (fp16, N=512) into
    4 PSUM banks, PSUM DMA'd straight to HBM.

Numerics (vs f64 reference, measured in emulation): max err / max|out|
~2.5e-3 -- fp16 matmul inputs, fp32 PSUM accumulation.
"""

import numpy as np

ORDER = 7
ALPHA = 1.0
BETA = 1.0
B_FULL, I_DIM, O_DIM = 4096, 512, 512
N_CORES = 8
BS = B_FULL // N_CORES  # 512 batch rows per core
P = 128                 # SBUF partitions
IC = I_DIM // P         # 4 i-chunks
BT = BS // P            # 4 batch tiles per core


def _recurrence_constants():
    """K1/K3 per reference, rescaled so q_d = t*q_{d-1} - g_d*q_{d-2}."""
    k1 = np.zeros(ORDER + 1, dtype=np.float64)
    k3 = np.zeros(ORDER + 1, dtype=np.float64)
    a, b = ALPHA, BETA
    for i in range(2, ORDER + 1):
        k1[i] = (2 * i + a + b) * (2 * i + a + b - 1) / (2 * i * (i + a + b))
        k3[i] = (
            (i + a - 1) * (i + b - 1) * (2 * i + a + b)
            / (i * (i + a + b) * (2 * i + a + b - 2))
        )
    s = np.zeros(ORDER + 1, dtype=np.float64)
    s[0] = 1.0
    s[1] = 0.5 * (a + b + 2.0)  # p_1 = s_1 * t  (the -(a-b)/2 term is 0)
    for d in range(2, ORDER + 1):
        s[d] = k1[d] * s[d - 1]
    g = np.zeros(ORDER + 1, dtype=np.float64)
    for d in range(2, ORDER + 1):
        g[d] = k3[d] * s[d - 2] / s[d]
    return s, g


_S, _G = _recurrence_constants()

_NC_CACHE = {}


def _build_bass():
    from contextlib import ExitStack
    from concourse import bacc, bass, tile, mybir

    nc = bacc.Bacc(
        "TRN2",
        target_bir_lowering=False,
        debug=False,
        num_devices=1,
    )
    f32 = mybir.dt.float32
    f16 = mybir.dt.float16

    xT = nc.dram_tensor("xT", [I_DIM, BS], f32, kind="ExternalInput")
    cf = nc.dram_tensor("cf", [ORDER, I_DIM, O_DIM], f16, kind="ExternalInput")
    out = nc.dram_tensor("out", [BS, O_DIM], f32, kind="ExternalOutput")

    FD = IC * BS  # 2048: all 4 i-chunks side by side on the free axis

    with tile.TileContext(nc) as tc, ExitStack() as ctx:
        pool = ctx.enter_context(tc.tile_pool(name="main", bufs=1))
        psum = ctx.enter_context(
            tc.tile_pool(name="psum", bufs=1, space=bass.MemorySpace.PSUM)
        )

        # x in (one DMA), tanh -> fp16 (one ACT op over [128, 4, 512])
        xt = pool.tile([P, IC, BS], f32, tag="x")
        nc.sync.dma_start(xt[:], xT.rearrange("(ic p) b -> p ic b", p=P))
        t = pool.tile([P, IC, BS], f16, tag="t")
        nc.scalar.activation(t[:], xt[:], mybir.ActivationFunctionType.Tanh)

        # coef in: one DMA per order d (cf[di] holds order d = di+1)
        cfs = [None] * (ORDER + 1)
        for d in range(1, ORDER + 1):
            c_t = pool.tile([P, IC, O_DIM], f16, tag=f"cf{d}")
            nc.gpsimd.dma_start(
                c_t[:], cf[d - 1].rearrange("(ic p) j -> p ic j", p=P)
            )
            cfs[d] = c_t

        # recurrence chain over the full [128, 2048] plane:
        #   q_1 = t; q_2 = t*t - g_2; q_d = t*q_{d-1} - g_d*q_{d-2}
        q = [None] * (ORDER + 1)
        q[1] = t
        for d in range(2, ORDER + 1):
            m = pool.tile([P, IC, BS], f16, tag=f"m{d}")
            nc.vector.tensor_tensor(m[:], t[:], q[d - 1][:], mybir.AluOpType.mult)
            qd = pool.tile([P, IC, BS], f16, tag=f"q{d}")
            if d == 2:
                # q_0 == 1: plain scalar add, no ones tile needed
                nc.vector.tensor_scalar_add(qd[:], m[:], -float(_G[d]))
            else:
                nc.vector.scalar_tensor_tensor(
                    qd[:],
                    q[d - 2][:],
                    -float(_G[d]),
                    m[:],
                    op0=mybir.AluOpType.mult,
                    op1=mybir.AluOpType.add,
                )
            q[d] = qd

        # matmuls: psum[b] += q[d][:, ic*BS+b*128 :+128].T @ cfs[d][:, ic*O :+O]
        ps = [
            psum.tile([P, O_DIM], f32, tag=f"ps{b}", name=f"ps{b}")
            for b in range(BT)
        ]
        for d in range(1, ORDER + 1):
            for ic in range(IC):
                first = d == 1 and ic == 0
                last = d == ORDER and ic == IC - 1
                for b in range(BT):
                    nc.tensor.matmul(
                        ps[b][:],
                        q[d][:, ic, b * P:(b + 1) * P],
                        cfs[d][:, ic, :],
                        start=first,
                        stop=last,
                    )

        # PSUM -> SBUF -> HBM (DMA cannot read PSUM directly)
        ot = pool.tile([P, BT, O_DIM], f32, tag="o")
        for b in range(BT):
            nc.scalar.copy(ot[:, b, :], ps[b][:])
        nc.sync.dma_start(
            out.rearrange("(bt p) j -> p bt j", p=P), ot[:]
        )

    nc.compile()
    return nc


def _get_nc():
    if "nc" not in _NC_CACHE:
        _NC_CACHE["nc"] = _build_bass()
    return _NC_CACHE["nc"]


def _host_prep(x, coef):
    """Shard + transform inputs. Returns (in_maps, bias)."""
    x = np.asarray(x, dtype=np.float32)
    coef = np.asarray(coef, dtype=np.float32)

    # [d, i, j] with the recurrence scale folded in, orders 1..7, fp16
    cf_t = coef.astype(np.float64).transpose(2, 1, 0)  # [8, I, O]
    cf_dev = np.ascontiguousarray(
        (cf_t[1:] * _S[1:, None, None]).astype(np.float16)
    )
    # d = 0 term: P_0 == 1  ->  bias[j] = sum_i coef[j, i, 0]
    bias = cf_t[0].sum(axis=0)  # [O] f64

    xT = np.ascontiguousarray(x.T)  # [I, B]
    in_maps = [
        {"xT": np.ascontiguousarray(xT[:, c * BS:(c + 1) * BS]), "cf": cf_dev}
        for c in range(N_CORES)
    ]
    return in_maps, bias


def kernel(x, coef):
    from concourse.bass_utils import run_bass_kernel_spmd

    nc = _get_nc()
    in_maps, bias = _host_prep(x, coef)
    res = run_bass_kernel_spmd(nc, in_maps, core_ids=list(range(N_CORES)))
    out = np.concatenate(
        [res.results[c]["out"] for c in range(N_CORES)], axis=0
    ).astype(np.float64)
    out += bias[None, :]
    return out.astype(np.float32)
